# revision 12
# baseline (speedup 1.0000x reference)
"""CBOW negative-sampling loss on 8 Trainium2 NeuronCores.

Iteration-1 fallback (verified 2785321 ns on HW): f32 tables, NG=1024,
4 SWDGE queues, DVE multiply+reduce dots. Restore over kernel.py if the
bf16 variant regresses.
"""

import os
import sys

import numpy as np

if "/opt/trn_rl_repo" not in sys.path:
    sys.path.insert(0, "/opt/trn_rl_repo")

VOCAB = 200000
D = 128
B = 32768
C = 8
K = 5
NCORES = 8
P = 128

GB = B // NCORES            # groups per core (4096)
W = 32768                   # vocab window size (int16 range)
NW = (VOCAB + W - 1) // W   # 7 windows
NG = 1024                   # indices per dma_gather
SEG = NG // P               # 8 rows per partition per chunk

NEG_N = GB * C * K          # 163840
CTX_N = GB * C              # 32768
POS_N = GB                  # 4096


def _chunks_for(total):
    out = []
    for w in range(NW):
        frac = (min(VOCAB, (w + 1) * W) - w * W) / VOCAB
        mean = total * frac
        sd = (total * frac * (1 - frac)) ** 0.5
        out.append(max(1, int(np.ceil((mean + 8 * sd) / NG))))
    return out


NEG_CH = _chunks_for(NEG_N)     # per-window chunk counts
CTX_CH = _chunks_for(CTX_N)
POS_CH = _chunks_for(POS_N)
NEG_TOT = sum(NEG_CH)
CTX_TOT = sum(CTX_CH)
POS_TOT = sum(POS_CH)
TOT_CH = NEG_TOT + CTX_TOT + POS_TOT

VROWS = 4224                # V table rows: 0..4095 real, 4096 zero, 4223 dump
VZERO = 4096
VDUMP = 4223

_CACHE = {}


def _build_program():
    import concourse.bass as bass
    import concourse.mybir as mybir
    from concourse import bacc
    from concourse.library_config import mlp
    from concourse.tile import TileContext

    f32 = mybir.dt.float32
    i16 = mybir.dt.int16

    nc = bacc.Bacc("TRN2", num_swdge_queues=4)
    uw = nc.declare_dram_parameter("u_weights", [VOCAB, D], f32, isOutput=False)
    vw = nc.declare_dram_parameter("v_weights", [VOCAB, D], f32, isOutput=False)
    idxd = nc.declare_dram_parameter(
        "idx_all", [P, TOT_CH * 2 * (NG // 16)], i16, isOutput=False
    )
    lp = nc.declare_dram_parameter("loss_parts", [P, 2], f32, isOutput=True)

    IW = NG // 16
    NCOMP = NEG_TOT + POS_TOT
    NZ = VROWS // P

    with TileContext(nc) as tc:
        with (
            tc.tile_pool(name="fix", bufs=1) as fixp,
            tc.tile_pool(name="vtd", bufs=1, space="DRAM") as vtp,
            tc.tile_pool(name="ct", bufs=6) as ctp,
            tc.tile_pool(name="eb", bufs=6) as ebp,
            tc.tile_pool(name="sm", bufs=6) as smp,
        ):
            nc.gpsimd.load_library(mlp)
            vtab_t = vtp.tile([VROWS, D], f32)
            vtab = vtab_t[:]
            idxt = fixp.tile([P, TOT_CH * 2 * IW], i16)
            nc.sync.dma_start(out=idxt[:], in_=idxd[:])
            zt = fixp.tile([P, NZ * D], f32)
            nc.vector.memset(zt[:], 0.0)
            acc = fixp.tile([P, 2], f32)
            nc.vector.memset(acc[:], 0.0)

            def idx_ap(chunk_i, which):
                off = (chunk_i * 2 + which) * IW
                return idxt[:, off:off + IW]

            # zero the V table (Tile tracks the DRAM pool tile deps)
            for i in range(NZ):
                nc.gpsimd.dma_start(
                    out=vtab[i * P:(i + 1) * P, :],
                    in_=zt[:, i * D:(i + 1) * D],
                )

            ch = 0
            # context phase: gather rows, scatter-add into vtab by group id
            for kk in range(CTX_TOT):
                et = ctp.tile([P, SEG, D], f32, tag="ce")
                nc.gpsimd.dma_gather(
                    et[:], vw[:], idx_ap(ch, 0), NG, NG, D,
                    queue_num=1 + kk % 3,
                )
                nc.gpsimd.dma_scatter_add(
                    vtab, et[:], idx_ap(ch, 1), NG, NG, D,
                )
                ch += 1

            # negatives then positives; vex gathers read vtab -> gate on sc
            for k in range(NCOMP):
                is_pos = k >= NEG_TOT
                src = uw if is_pos else vw
                et = ebp.tile([P, SEG, D], f32, tag="emb")
                vt = ebp.tile([P, SEG, D], f32, tag="vex")
                nc.gpsimd.dma_gather(
                    et[:], src[:], idx_ap(ch, 0), NG, NG, D,
                    queue_num=(2 * k) % 4,
                )
                nc.gpsimd.dma_gather(
                    vt[:], vtab, idx_ap(ch, 1), NG, NG, D,
                    queue_num=(2 * k + 1) % 4,
                )
                ch += 1

                st = smp.tile([P, SEG], f32, tag="sco")
                nc.vector.tensor_tensor(
                    out=et[:], in0=et[:], in1=vt[:], op=mybir.AluOpType.mult,
                )
                nc.vector.tensor_reduce(
                    out=st[:], in_=et[:], axis=mybir.AxisListType.X,
                    op=mybir.AluOpType.add,
                )
                nc.vector.tensor_scalar(
                    out=st[:], in0=st[:], scalar1=10.0, scalar2=-10.0,
                    op0=mybir.AluOpType.min, op1=mybir.AluOpType.max,
                )
                spt = smp.tile([P, SEG], f32, tag="sp")
                bt = smp.tile([P, 1], f32, tag="blk")
                nc.scalar.activation(
                    out=spt[:], in_=st[:],
                    func=mybir.ActivationFunctionType.Exp,
                    scale=-1.0 if is_pos else 1.0,
                )
                nc.scalar.activation(
                    out=spt[:], in_=spt[:],
                    func=mybir.ActivationFunctionType.Ln, bias=1.0,
                    accum_out=bt[:],
                )
                col = 0 if is_pos else 1
                nc.vector.tensor_tensor(
                    out=acc[:, col:col + 1], in0=acc[:, col:col + 1],
                    in1=bt[:], op=mybir.AluOpType.add,
                )

            nc.sync.dma_start(out=lp[:], in_=acc[:])
    nc.finalize()
    return nc


def _window_sort(idx, gid, chunks):
    """Sort (idx, gid) by vocab window; pad each window to chunks[w]*NG.
    Returns wrapped int16 arrays [nch, P, NG//16] x2 and the pad count."""
    order = np.argsort(idx // W, kind="stable")
    si, sg = idx[order], gid[order]
    wi = si // W
    out_i, out_g = [], []
    npad = 0
    for w in range(NW):
        m = wi == w
        li = (si[m] - w * W).astype(np.int16)
        lg = sg[m].astype(np.int16)
        cap = chunks[w] * NG
        if len(li) > cap:
            raise RuntimeError(f"window {w} overflow: {len(li)} > {cap}")
        pad = cap - len(li)
        npad += pad
        li = np.concatenate([li, np.zeros(pad, np.int16)])
        lg = np.concatenate([lg, np.full(pad, VZERO, np.int16)])
        out_i.append(li)
        out_g.append(lg)
    fi = np.concatenate(out_i).reshape(-1, NG)
    fg = np.concatenate(out_g).reshape(-1, NG)

    def wrap(a):  # [nch, NG] -> [nch, P, NG//16]
        w16 = a.reshape(a.shape[0], NG // 16, 16)
        return np.tile(w16.transpose(0, 2, 1), (1, 8, 1)).astype(np.int16)

    return wrap(fi), wrap(fg), npad


def _prep_core(pos_u, pos_v, neg_v, core):
    sl = slice(core * GB, (core + 1) * GB)
    gids = np.arange(GB, dtype=np.int64)
    negf = neg_v.reshape(B, C * K)[sl].astype(np.int64)
    ctxf = pos_v.reshape(B, C)[sl].astype(np.int64)
    posf = pos_u.reshape(B, C)[sl][:, 0].astype(np.int64)

    neg_i, neg_g, npad_n = _window_sort(
        negf.ravel(), np.repeat(gids, C * K), NEG_CH)
    ctx_i, ctx_g, _ = _window_sort(
        ctxf.ravel(), np.repeat(gids, C), CTX_CH)
    # ctx companion is the scatter target: pads go to the dump row
    ctx_g[ctx_g == VZERO] = VDUMP
    pos_i, pos_g, npad_p = _window_sort(posf, gids, POS_CH)

    # interleave [idx, gid] per chunk in program order: ctx, neg, pos
    parts = []
    for i_arr, g_arr in ((ctx_i, ctx_g), (neg_i, neg_g), (pos_i, pos_g)):
        inter = np.empty((i_arr.shape[0] * 2, P, NG // 16), np.int16)
        inter[0::2] = i_arr
        inter[1::2] = g_arr
        parts.append(inter)
    allc = np.concatenate(parts, axis=0)            # [TOT_CH*2, P, 64]
    packed = np.ascontiguousarray(
        allc.transpose(1, 0, 2).reshape(P, TOT_CH * 2 * (NG // 16))
    )
    return packed, npad_n, npad_p


def _prep_indices(pos_u, pos_v, neg_v):
    out = []
    for c in range(NCORES):
        out.append(_prep_core(pos_u, pos_v, neg_v, c))
    return out


def _prep_tables(u_weights, v_weights):
    u_w = np.ascontiguousarray(np.asarray(u_weights, dtype=np.float32))
    v_w = np.ascontiguousarray(np.asarray(v_weights, dtype=np.float32))
    return u_w, v_w


def kernel(u_weights, v_weights, pos_u, pos_v, neg_v, context_size):
    from concourse.bass_utils import run_bass_kernel_spmd

    assert int(context_size) == C
    u_w, v_w = _prep_tables(u_weights, v_weights)
    pos_u = np.asarray(pos_u)
    pos_v = np.asarray(pos_v)
    neg_v = np.asarray(neg_v)

    if "nc" not in _CACHE:
        _CACHE["nc"] = _build_program()
    nc = _CACHE["nc"]

    prep = _prep_indices(pos_u, pos_v, neg_v)
    in_maps = [
        {"u_weights": u_w, "v_weights": v_w, "idx_all": prep[c][0]}
        for c in range(NCORES)
    ]
    res = run_bass_kernel_spmd(nc, in_maps, list(range(NCORES)))
    LN2 = float(np.log(2.0))
    total = np.float64(0.0)
    for c in range(NCORES):
        parts = res.results[c]["loss_parts"].astype(np.float64)
        pos_sum = parts[:, 0].sum() - prep[c][2] * LN2
        neg_sum = parts[:, 1].sum() - prep[c][1] * LN2
        total += pos_sum + neg_sum / (C * K)
    return np.float32(total / B)
